# revision 10
# baseline (speedup 1.0000x reference)
"""Trainium2 Bass kernel for nn_AttentionBlock (B=8, C=128, H=W=64).

Data-parallel over batch across 8 NeuronCores (one batch element per core).
Per core, full 4096x4096 single-head attention:

  xt = x.T;  q = xt@(qw*scale) + qb*scale;  k = xt@kw;  v0 = xt@vw
  S = q k^T;  P = softmax(S);  out = xt + P@v0@pw + (vb@pw + pb)
  (k-bias is softmax-invariant; v-bias folds through rowsum==1)

v3 dataflow:
  - P row-tiles [128, 4096] fp16 are transposed by the DMA xbar
    (dma_start(transpose=True) on the sync HWDGE ring, ~4.9us per 1MB
    tile) straight into the [p, mblock, n] layout the PV matmuls
    consume.  The PE runs no transpose-mode instructions, so the HAM
    clock gate stays warm (2.4 GHz) for the real matmuls.
  - software pipelining: the PV matmuls of group g-1 are emitted
    between the S-chunk matmuls of group g, so the PE never idles
    during the softmax phase and DVE/ACT never idle during PV.
  - S chunks accumulate in PSUM (3 rotating 1024-wide bufs); DVE
    negated max per chunk; ACT exp straight from PSUM with accum_out
    rowsums.  Chunks 0-2 exp immediately with their chunk-local max
    (only chunk 3 waits for the global max), and the fixups are split
    2-on-DVE / 1-on-ACT.
  - H^T computed directly by four 128-col matmuls (lhsT = O_sb
    128-slices, rhs = pw) instead of proj + transpose, so the sync
    ring carries only P transposes (no head-of-line blocking).
"""

import numpy as np

C = 128
N = 4096  # tokens per batch element (64*64)
B = 8
H = W = 64

_cache = {}


def _build(n_tokens=N):
    import concourse.bass as bass
    import concourse.mybir as mybir
    import concourse.tile as tile
    from concourse import bacc

    f32 = mybir.dt.float32
    f16 = mybir.dt.float16
    Alu = mybir.AluOpType
    Act = mybir.ActivationFunctionType

    NTOK = n_tokens
    NTILES = NTOK // 128          # q-token row tiles
    MCHUNK = 1024                 # S psum chunk width (2 banks)
    MCH_CNT = NTOK // MCHUNK      # chunks per row-tile
    GRP = min(512, NTOK)          # PV n-group size
    TPG = GRP // 128              # row-tiles per group
    NGRP = NTOK // GRP
    MBLK = NTOK // 128            # m blocks total
    NDEF = 2                      # chunks exp'd with the true row max
    NCOR = MCH_CNT - NDEF         # chunks exp'd early w/ local max
    PVT = MBLK // TPG             # PV matmuls interleaved per row-tile

    nc = bacc.Bacc("TRN2", target_bir_lowering=False, debug=False, num_devices=8)

    xh_e = nc.dram_tensor("xh", [C, NTOK], f16, kind="ExternalInput")
    xpb_e = nc.dram_tensor("xpb", [NTOK, C], f32, kind="ExternalInput")
    qw_e = nc.dram_tensor("qw", [C, C], f16, kind="ExternalInput")
    kw_e = nc.dram_tensor("kw", [C, C], f16, kind="ExternalInput")
    vw_e = nc.dram_tensor("vw", [C, C], f16, kind="ExternalInput")
    pw_e = nc.dram_tensor("pw", [C, C], f16, kind="ExternalInput")
    qb_e = nc.dram_tensor("qb", [C, 1], f32, kind="ExternalInput")
    out_e = nc.dram_tensor("out", [NTOK, C], f32, kind="ExternalOutput")

    with tile.TileContext(nc) as tc:
        with (
            tc.tile_pool(name="persist", bufs=1) as persist,
            tc.tile_pool(name="pp", bufs=3) as p_pool,
            tc.tile_pool(name="ptp", bufs=2) as pt_pool,
            tc.tile_pool(name="small", bufs=4) as small,
            tc.tile_pool(name="stats", bufs=8) as stats,
            tc.tile_pool(name="ivp", bufs=2 * TPG + 2) as ivp,
            tc.tile_pool(name="psS", bufs=3, space="PSUM") as psS,
            tc.tile_pool(name="psA", bufs=1, space="PSUM") as psA,
            tc.tile_pool(name="psB", bufs=1, space="PSUM") as psB,
        ):
            # ---- constants / weights ----
            qw_sb = persist.tile([C, C], f16, tag="qw")
            kw_sb = persist.tile([C, C], f16, tag="kw")
            vw_sb = persist.tile([C, C], f16, tag="vw")
            pw_sb = persist.tile([C, C], f16, tag="pw")
            qb_sb = persist.tile([C, 1], f32, tag="qb")
            nc.gpsimd.dma_start(out=qw_sb[:], in_=qw_e[:])
            nc.gpsimd.dma_start(out=kw_sb[:], in_=kw_e[:])
            nc.gpsimd.dma_start(out=vw_sb[:], in_=vw_e[:])
            nc.gpsimd.dma_start(out=pw_sb[:], in_=pw_e[:])
            nc.gpsimd.dma_start(out=qb_sb[:], in_=qb_e[:])

            xh_sb = persist.tile([C, NTOK], f16, tag="xh")
            nc.gpsimd.dma_start(out=xh_sb[:], in_=xh_e[:])

            # ---- QT / KT (c_out, n) fp16 ----
            QT = persist.tile([C, NTOK], f16, tag="QT")
            KT = persist.tile([C, NTOK], f16, tag="KT")
            for j in range(NTOK // 512):
                sl = slice(j * 512, (j + 1) * 512)
                pq = psA.tile([C, 512], f32, tag="a")
                nc.tensor.matmul(pq[:], lhsT=qw_sb[:], rhs=xh_sb[:, sl])
                nc.vector.tensor_scalar(
                    out=QT[:, sl], in0=pq[:], scalar1=qb_sb[:], scalar2=None,
                    op0=Alu.add,
                )
                pk = psB.tile([C, 512], f32, tag="b")
                nc.tensor.matmul(pk[:], lhsT=kw_sb[:], rhs=xh_sb[:, sl])
                nc.scalar.activation(out=KT[:, sl], in_=pk[:], func=Act.Copy)

            # ---- V in (m, c) layout: V[i*128+p, c] at V_sb[p, i, c] ----
            V_sb = persist.tile([C, MBLK, 128], f16, tag="V")
            for i in range(MBLK):
                pv = psB.tile([C, 512], f32, tag="b")
                nc.tensor.matmul(
                    pv[:, :128], lhsT=xh_sb[:, i * 128:(i + 1) * 128],
                    rhs=vw_sb[:],
                )
                nc.scalar.activation(out=V_sb[:, i, :], in_=pv[:, :128],
                                     func=Act.Copy)

            iv_tiles = [None] * NTILES
            prev = None  # (g, PT, O_ps) of the group whose PV is in flight

            def emit_pv(prevst, mb0, mb1):
                gp, PTp, O_ps = prevst
                for mb in range(mb0, mb1):
                    nc.tensor.matmul(
                        O_ps[:], lhsT=V_sb[:, mb, :],
                        rhs=PTp[:, :, mb, :],
                        start=(mb == 0), stop=(mb == MBLK - 1),
                    )

            def emit_tail(prevst):
                # O drain, H^T matmuls, normalize+residual, store
                gp, PTp, O_ps = prevst
                O_sb = small.tile([C, GRP], f16, tag="O")
                nc.scalar.activation(out=O_sb[:], in_=O_ps[:], func=Act.Copy)
                Hps = psB.tile([C, TPG, 128], f32, tag="b")
                for t in range(TPG):
                    nc.tensor.matmul(
                        Hps[:, t, :], lhsT=O_sb[:, t * 128:(t + 1) * 128],
                        rhs=pw_sb[:],
                    )
                xpb_g = small.tile([C, TPG, 128], f32, tag="xpb")
                nc.gpsimd.dma_start(
                    out=xpb_g[:],
                    in_=xpb_e[gp * GRP:(gp + 1) * GRP, :].rearrange(
                        "(t p) c -> p t c", p=128),
                )
                out_g = small.tile([C, TPG, 128], f32, tag="og")
                for t in range(TPG):
                    nt = gp * TPG + t
                    nc.vector.scalar_tensor_tensor(
                        out=out_g[:, t, :],
                        in0=Hps[:, t, :],
                        scalar=iv_tiles[nt][:], in1=xpb_g[:, t, :],
                        op0=Alu.mult, op1=Alu.add,
                    )
                nc.gpsimd.dma_start(
                    out=out_e[gp * GRP:(gp + 1) * GRP, :].rearrange(
                        "(t p) c -> p t c", p=128),
                    in_=out_g[:],
                )

            for g in range(NGRP):
                PT_g = pt_pool.tile([C, TPG, MBLK, 128], f16, tag="PT")

                for t in range(TPG):
                    nt = g * TPG + t
                    qsl = slice(nt * 128, (nt + 1) * 128)
                    nm = stats.tile([C, MCH_CNT], f32, tag="nm")
                    rsc = stats.tile([C, MCH_CNT], f32, tag="rsc")
                    P_t = p_pool.tile([C, NTOK], f16, tag="P")
                    sps_defer = []
                    for h in range(MCH_CNT):
                        sps = psS.tile([C, MCHUNK], f32, tag="s")
                        for q in range(MCHUNK // 512):
                            nc.tensor.matmul(
                                sps[:, q * 512:(q + 1) * 512],
                                lhsT=QT[:, qsl],
                                rhs=KT[:, h * MCHUNK + q * 512:
                                       h * MCHUNK + (q + 1) * 512],
                            )
                        if h == MCH_CNT - 2 and prev is not None:
                            # slot half the PV matmuls behind chunk 2's mms
                            emit_pv(prev, t * PVT, t * PVT + PVT // 2)
                        nc.vector.tensor_reduce(
                            out=nm[:, h:h + 1], in_=sps[:],
                            axis=mybir.AxisListType.X, op=Alu.max,
                            negate=True,
                        )
                        if h < NCOR:
                            # early exp with chunk-local max, fixed up later
                            nc.scalar.activation(
                                out=P_t[:, h * MCHUNK:(h + 1) * MCHUNK],
                                in_=sps[:], func=Act.Exp,
                                bias=nm[:, h:h + 1], scale=1.0,
                                accum_out=rsc[:, h:h + 1],
                            )
                        else:
                            sps_defer.append(sps)
                    if prev is not None:
                        emit_pv(prev, t * PVT + PVT // 2, (t + 1) * PVT)
                    # global row max M = -min(nm)
                    ngm = stats.tile([C, 1], f32, tag="ngm")
                    nc.vector.tensor_reduce(
                        out=ngm[:], in_=nm[:], axis=mybir.AxisListType.X,
                        op=Alu.min,
                    )
                    # deferred chunks use the exact bias — no correction
                    for k, sps in enumerate(sps_defer):
                        h = NCOR + k
                        nc.scalar.activation(
                            out=P_t[:, h * MCHUNK:(h + 1) * MCHUNK],
                            in_=sps[:], func=Act.Exp,
                            bias=ngm[:], scale=1.0,
                            accum_out=rsc[:, h:h + 1],
                        )
                    # corr_h = exp(cm_h - M); rescale early chunks + rsc
                    corrs = stats.tile([C, NCOR], f32, tag="corrs")
                    nc.scalar.activation(
                        out=corrs[:], in_=nm[:, 0:NCOR], func=Act.Exp,
                        bias=ngm[:], scale=-1.0,
                    )
                    nc.vector.tensor_tensor(
                        out=rsc[:, 0:NCOR], in0=rsc[:, 0:NCOR],
                        in1=corrs[:], op=Alu.mult,
                    )
                    for h in range(NCOR):
                        csl = slice(h * MCHUNK, (h + 1) * MCHUNK)
                        nc.vector.tensor_scalar(
                            out=P_t[:, csl], in0=P_t[:, csl],
                            scalar1=corrs[:, h:h + 1], scalar2=None,
                            op0=Alu.mult,
                        )
                    rs = stats.tile([C, 1], f32, tag="rs")
                    nc.vector.tensor_reduce(
                        out=rs[:], in_=rsc[:], axis=mybir.AxisListType.X,
                        op=Alu.add,
                    )
                    iv = ivp.tile([C, 1], f32, tag="iv")
                    nc.vector.reciprocal(iv[:], rs[:])
                    iv_tiles[nt] = iv

                    # xbar block transpose: PT_g[p, t, mb, n] = P_t[n, mb*128+p]
                    nc.sync.dma_start(out=PT_g[:, t], in_=P_t[:],
                                      transpose=True)

                if prev is not None:
                    emit_tail(prev)
                O_ps = psA.tile([C, GRP], f32, tag="a")
                prev = (g, PT_g, O_ps)

            emit_pv(prev, 0, MBLK)
            emit_tail(prev)

    nc.compile()
    return nc


def _get_nc(n_tokens=N):
    if n_tokens not in _cache:
        _cache[n_tokens] = _build(n_tokens)
    return _cache[n_tokens]


def prep_inputs(x, qw, qb, kw, kb, vw, vb, proj_w, proj_b, n_tokens=N):
    """Host-side prep: shard over batch, fold scale/biases, transpose."""
    x = np.asarray(x, dtype=np.float32)
    b, c, h, w = x.shape
    scale = c ** (-0.5)
    qw_s = (np.asarray(qw, np.float32) * scale).astype(np.float16)
    kw16 = np.asarray(kw, np.float32).astype(np.float16)
    vw16 = np.asarray(vw, np.float32).astype(np.float16)
    pw16 = np.asarray(proj_w, np.float32).astype(np.float16)
    qb_s = (np.asarray(qb, np.float32) * scale).reshape(c, 1).astype(np.float32)
    pb2 = (np.asarray(vb, np.float32) @ np.asarray(proj_w, np.float32)
           + np.asarray(proj_b, np.float32)).astype(np.float32)

    in_maps = []
    for i in range(b):
        xc = x[i].reshape(c, h * w)[:, :n_tokens]
        xt = xc.T.copy()
        in_maps.append({
            "xh": np.ascontiguousarray(xc).astype(np.float16),
            "xpb": np.ascontiguousarray(xt + pb2[None, :]),
            "qw": qw_s, "kw": kw16, "vw": vw16, "pw": pw16,
            "qb": qb_s,
        })
    return in_maps


def kernel(x, qw, qb, kw, kb, vw, vb, proj_w, proj_b, _trace=False):
    from concourse.bass_utils import run_bass_kernel_spmd

    nc = _get_nc(N)
    in_maps = prep_inputs(x, qw, qb, kw, kb, vw, vb, proj_w, proj_b)
    res = run_bass_kernel_spmd(nc, in_maps, core_ids=list(range(B)),
                               trace=_trace)
    kernel.last_results = res
    out = np.stack([np.asarray(res.results[i]["out"]) for i in range(B)])
    return out.reshape(B, H, W, C).astype(np.float32)


# revision 14
# speedup vs baseline: 1.0366x; 1.0366x over previous
"""Trainium2 Bass kernel for nn_AttentionBlock (B=8, C=128, H=W=64).

Data-parallel over batch across 8 NeuronCores (one batch element per core).
Per core, full 4096x4096 single-head attention:

  xt = x.T;  q = xt@(qw*scale) + qb*scale;  k = xt@kw;  v0 = xt@vw
  S = q k^T;  P = softmax(S);  out = xt + P@v0@pw + (vb@pw + pb)
  (k-bias is softmax-invariant; v-bias folds through rowsum==1)

v3 dataflow:
  - P row-tiles [128, 4096] fp16 are transposed by the DMA xbar
    (dma_start(transpose=True) on the sync HWDGE ring, ~4.9us per 1MB
    tile) straight into the [p, mblock, n] layout the PV matmuls
    consume.  The PE runs no transpose-mode instructions, so the HAM
    clock gate stays warm (2.4 GHz) for the real matmuls.
  - software pipelining: the PV matmuls of group g-1 are emitted
    between the S-chunk matmuls of group g, so the PE never idles
    during the softmax phase and DVE/ACT never idle during PV.
  - S chunks accumulate in PSUM (3 rotating 1024-wide bufs); DVE
    negated max per chunk; ACT exp straight from PSUM with accum_out
    rowsums.  Chunks 0-2 exp immediately with their chunk-local max
    (only chunk 3 waits for the global max), and the fixups are split
    2-on-DVE / 1-on-ACT.
  - H^T computed directly by four 128-col matmuls (lhsT = O_sb
    128-slices, rhs = pw) instead of proj + transpose, so the sync
    ring carries only P transposes (no head-of-line blocking).
"""

import numpy as np

C = 128
N = 4096  # tokens per batch element (64*64)
B = 8
H = W = 64

_cache = {}


def _build(n_tokens=N):
    import concourse.bass as bass
    import concourse.mybir as mybir
    import concourse.tile as tile
    from concourse import bacc

    f32 = mybir.dt.float32
    f16 = mybir.dt.float16
    Alu = mybir.AluOpType
    Act = mybir.ActivationFunctionType

    NTOK = n_tokens
    NTILES = NTOK // 128          # q-token row tiles
    MCHUNK = 1024                 # S psum chunk width (2 banks)
    MCH_CNT = NTOK // MCHUNK      # chunks per row-tile
    GRP = min(512, NTOK)          # PV n-group size
    TPG = GRP // 128              # row-tiles per group
    NGRP = NTOK // GRP
    MBLK = NTOK // 128            # m blocks total
    NDEF = 2                      # chunks exp'd with the true row max
    NCOR = MCH_CNT - NDEF         # chunks exp'd early w/ local max
    PVT = MBLK // TPG             # PV matmuls interleaved per row-tile

    nc = bacc.Bacc("TRN2", target_bir_lowering=False, debug=False, num_devices=8)

    xh_e = nc.dram_tensor("xh", [C, NTOK], f16, kind="ExternalInput")
    xpb_e = nc.dram_tensor("xpb", [NTOK, C], f32, kind="ExternalInput")
    qw_e = nc.dram_tensor("qw", [C, C], f16, kind="ExternalInput")
    kw_e = nc.dram_tensor("kw", [C, C], f16, kind="ExternalInput")
    vw_e = nc.dram_tensor("vw", [C, C], f16, kind="ExternalInput")
    pw_e = nc.dram_tensor("pw", [C, C], f16, kind="ExternalInput")
    qb_e = nc.dram_tensor("qb", [C, 1], f32, kind="ExternalInput")
    out_e = nc.dram_tensor("out", [NTOK, C], f32, kind="ExternalOutput")

    with tile.TileContext(nc) as tc:
        with (
            tc.tile_pool(name="persist", bufs=1) as persist,
            tc.tile_pool(name="pp", bufs=3) as p_pool,
            tc.tile_pool(name="ptp", bufs=2) as pt_pool,
            tc.tile_pool(name="small", bufs=4) as small,
            tc.tile_pool(name="stats", bufs=12) as stats,
            tc.tile_pool(name="ivp", bufs=2 * TPG + 2) as ivp,
            tc.tile_pool(name="psS", bufs=3, space="PSUM") as psS,
            tc.tile_pool(name="psA", bufs=1, space="PSUM") as psA,
            tc.tile_pool(name="psB", bufs=1, space="PSUM") as psB,
        ):
            # ---- constants / weights ----
            qw_sb = persist.tile([C, C], f16, tag="qw")
            kw_sb = persist.tile([C, C], f16, tag="kw")
            vw_sb = persist.tile([C, C], f16, tag="vw")
            pw_sb = persist.tile([C, C], f16, tag="pw")
            qb_sb = persist.tile([C, 1], f32, tag="qb")
            nc.gpsimd.dma_start(out=qw_sb[:], in_=qw_e[:])
            nc.gpsimd.dma_start(out=kw_sb[:], in_=kw_e[:])
            nc.gpsimd.dma_start(out=vw_sb[:], in_=vw_e[:])
            nc.gpsimd.dma_start(out=pw_sb[:], in_=pw_e[:])
            nc.gpsimd.dma_start(out=qb_sb[:], in_=qb_e[:])

            xh_sb = persist.tile([C, NTOK], f16, tag="xh")
            nc.gpsimd.dma_start(out=xh_sb[:], in_=xh_e[:])

            # ---- QT / KT (c_out, n) fp16 ----
            QT = persist.tile([C, NTOK], f16, tag="QT")
            KT = persist.tile([C, NTOK], f16, tag="KT")
            for j in range(NTOK // 512):
                sl = slice(j * 512, (j + 1) * 512)
                pq = psA.tile([C, 512], f32, tag="a")
                nc.tensor.matmul(pq[:], lhsT=qw_sb[:], rhs=xh_sb[:, sl])
                nc.vector.tensor_scalar(
                    out=QT[:, sl], in0=pq[:], scalar1=qb_sb[:], scalar2=None,
                    op0=Alu.add,
                )
                pk = psB.tile([C, 512], f32, tag="b")
                nc.tensor.matmul(pk[:], lhsT=kw_sb[:], rhs=xh_sb[:, sl])
                nc.scalar.activation(out=KT[:, sl], in_=pk[:], func=Act.Copy)

            # ---- V in (m, c) layout: V[i*128+p, c] at V_sb[p, i, c] ----
            V_sb = persist.tile([C, MBLK, 128], f16, tag="V")
            for i in range(MBLK):
                pv = psB.tile([C, 512], f32, tag="b")
                nc.tensor.matmul(
                    pv[:, :128], lhsT=xh_sb[:, i * 128:(i + 1) * 128],
                    rhs=vw_sb[:],
                )
                nc.scalar.activation(out=V_sb[:, i, :], in_=pv[:, :128],
                                     func=Act.Copy)

            iv_tiles = [None] * NTILES
            prev = None  # (g, PT, O_ps) of the group whose PV is in flight
            pend = None  # deferred softmax tail of the previous row-tile

            def emit_soft_tail(st):
                # corrections + rowsum + reciprocal + transpose for a tile,
                # deferred one tile so the ACT->DVE round trips hide behind
                # the next tile's max reduces
                nm, rsc, ngm, corrs, P_t, PT_dst, nt = st
                nc.vector.tensor_tensor(
                    out=rsc[:, 0:NCOR], in0=rsc[:, 0:NCOR],
                    in1=corrs[:], op=Alu.mult,
                )
                for h in range(NCOR):
                    csl = slice(h * MCHUNK, (h + 1) * MCHUNK)
                    nc.vector.tensor_scalar(
                        out=P_t[:, csl], in0=P_t[:, csl],
                        scalar1=corrs[:, h:h + 1], scalar2=None,
                        op0=Alu.mult,
                    )
                rs = stats.tile([C, 1], f32, tag="rs")
                nc.vector.tensor_reduce(
                    out=rs[:], in_=rsc[:], axis=mybir.AxisListType.X,
                    op=Alu.add,
                )
                iv = ivp.tile([C, 1], f32, tag="iv")
                nc.vector.reciprocal(iv[:], rs[:])
                iv_tiles[nt] = iv
                # xbar block transpose: PT[p, t, mb, n] = P_t[n, mb*128+p]
                nc.sync.dma_start(out=PT_dst, in_=P_t[:], transpose=True)

            def emit_pv(prevst, mb0, mb1):
                gp, PTp, O_ps = prevst
                for mb in range(mb0, mb1):
                    nc.tensor.matmul(
                        O_ps[:], lhsT=V_sb[:, mb, :],
                        rhs=PTp[:, :, mb, :],
                        start=(mb == 0), stop=(mb == MBLK - 1),
                    )

            def emit_tail(prevst):
                # O drain, H^T matmuls, normalize+residual, store
                gp, PTp, O_ps = prevst
                O_sb = small.tile([C, GRP], f16, tag="O")
                nc.scalar.activation(out=O_sb[:], in_=O_ps[:], func=Act.Copy)
                Hps = psB.tile([C, TPG, 128], f32, tag="b")
                for t in range(TPG):
                    nc.tensor.matmul(
                        Hps[:, t, :], lhsT=O_sb[:, t * 128:(t + 1) * 128],
                        rhs=pw_sb[:],
                    )
                xpb_g = small.tile([C, TPG, 128], f32, tag="xpb")
                nc.gpsimd.dma_start(
                    out=xpb_g[:],
                    in_=xpb_e[gp * GRP:(gp + 1) * GRP, :].rearrange(
                        "(t p) c -> p t c", p=128),
                )
                out_g = small.tile([C, TPG, 128], f32, tag="og")
                for t in range(TPG):
                    nt = gp * TPG + t
                    nc.vector.scalar_tensor_tensor(
                        out=out_g[:, t, :],
                        in0=Hps[:, t, :],
                        scalar=iv_tiles[nt][:], in1=xpb_g[:, t, :],
                        op0=Alu.mult, op1=Alu.add,
                    )
                nc.gpsimd.dma_start(
                    out=out_e[gp * GRP:(gp + 1) * GRP, :].rearrange(
                        "(t p) c -> p t c", p=128),
                    in_=out_g[:],
                )

            for g in range(NGRP):
                PT_g = pt_pool.tile([C, TPG, MBLK, 128], f16, tag="PT")

                for t in range(TPG):
                    nt = g * TPG + t
                    qsl = slice(nt * 128, (nt + 1) * 128)
                    nm = stats.tile([C, MCH_CNT], f32, tag="nm")
                    rsc = stats.tile([C, MCH_CNT], f32, tag="rsc")
                    P_t = p_pool.tile([C, NTOK], f16, tag="P")
                    sps_all = []
                    for h in range(MCH_CNT - 1):
                        sps = psS.tile([C, MCHUNK], f32, tag="s")
                        for q in range(MCHUNK // 512):
                            nc.tensor.matmul(
                                sps[:, q * 512:(q + 1) * 512],
                                lhsT=QT[:, qsl],
                                rhs=KT[:, h * MCHUNK + q * 512:
                                       h * MCHUNK + (q + 1) * 512],
                            )
                        sps_all.append(sps)
                    # previous tile's softmax tail: its ACT->DVE round trips
                    # overlap this tile's matmuls and max reduces
                    if pend is not None:
                        emit_soft_tail(pend)
                    if prev is not None:
                        emit_pv(prev, t * PVT, t * PVT + PVT // 2)
                    h = MCH_CNT - 1
                    sps = psS.tile([C, MCHUNK], f32, tag="s")
                    for q in range(MCHUNK // 512):
                        nc.tensor.matmul(
                            sps[:, q * 512:(q + 1) * 512],
                            lhsT=QT[:, qsl],
                            rhs=KT[:, h * MCHUNK + q * 512:
                                   h * MCHUNK + (q + 1) * 512],
                        )
                    sps_all.append(sps)
                    if prev is not None:
                        emit_pv(prev, t * PVT + PVT // 2, (t + 1) * PVT)
                    for h in range(MCH_CNT):
                        nc.vector.tensor_reduce(
                            out=nm[:, h:h + 1], in_=sps_all[h][:],
                            axis=mybir.AxisListType.X, op=Alu.max,
                            negate=True,
                        )
                        if h < NCOR:
                            # early exp with chunk-local max, fixed up later
                            nc.scalar.activation(
                                out=P_t[:, h * MCHUNK:(h + 1) * MCHUNK],
                                in_=sps_all[h][:], func=Act.Exp,
                                bias=nm[:, h:h + 1], scale=1.0,
                                accum_out=rsc[:, h:h + 1],
                            )
                    # global row max M = -min(nm)
                    ngm = stats.tile([C, 1], f32, tag="ngm")
                    nc.vector.tensor_reduce(
                        out=ngm[:], in_=nm[:], axis=mybir.AxisListType.X,
                        op=Alu.min,
                    )
                    # corr_h = exp(cm_h - M)
                    corrs = stats.tile([C, NCOR], f32, tag="corrs")
                    nc.scalar.activation(
                        out=corrs[:], in_=nm[:, 0:NCOR], func=Act.Exp,
                        bias=ngm[:], scale=-1.0,
                    )
                    # deferred chunks use the exact bias — no correction
                    for h in range(NCOR, MCH_CNT):
                        nc.scalar.activation(
                            out=P_t[:, h * MCHUNK:(h + 1) * MCHUNK],
                            in_=sps_all[h][:], func=Act.Exp,
                            bias=ngm[:], scale=1.0,
                            accum_out=rsc[:, h:h + 1],
                        )
                    pend = (nm, rsc, ngm, corrs, P_t, PT_g[:, t], nt)

                if prev is not None:
                    emit_tail(prev)
                O_ps = psA.tile([C, GRP], f32, tag="a")
                prev = (g, PT_g, O_ps)

            emit_soft_tail(pend)
            emit_pv(prev, 0, MBLK)
            emit_tail(prev)

    nc.compile()
    return nc


def _get_nc(n_tokens=N):
    if n_tokens not in _cache:
        _cache[n_tokens] = _build(n_tokens)
    return _cache[n_tokens]


def prep_inputs(x, qw, qb, kw, kb, vw, vb, proj_w, proj_b, n_tokens=N):
    """Host-side prep: shard over batch, fold scale/biases, transpose."""
    x = np.asarray(x, dtype=np.float32)
    b, c, h, w = x.shape
    scale = c ** (-0.5)
    qw_s = (np.asarray(qw, np.float32) * scale).astype(np.float16)
    kw16 = np.asarray(kw, np.float32).astype(np.float16)
    vw16 = np.asarray(vw, np.float32).astype(np.float16)
    pw16 = np.asarray(proj_w, np.float32).astype(np.float16)
    qb_s = (np.asarray(qb, np.float32) * scale).reshape(c, 1).astype(np.float32)
    pb2 = (np.asarray(vb, np.float32) @ np.asarray(proj_w, np.float32)
           + np.asarray(proj_b, np.float32)).astype(np.float32)

    in_maps = []
    for i in range(b):
        xc = x[i].reshape(c, h * w)[:, :n_tokens]
        xt = xc.T.copy()
        in_maps.append({
            "xh": np.ascontiguousarray(xc).astype(np.float16),
            "xpb": np.ascontiguousarray(xt + pb2[None, :]),
            "qw": qw_s, "kw": kw16, "vw": vw16, "pw": pw16,
            "qb": qb_s,
        })
    return in_maps


def kernel(x, qw, qb, kw, kb, vw, vb, proj_w, proj_b, _trace=False):
    from concourse.bass_utils import run_bass_kernel_spmd

    nc = _get_nc(N)
    in_maps = prep_inputs(x, qw, qb, kw, kb, vw, vb, proj_w, proj_b)
    res = run_bass_kernel_spmd(nc, in_maps, core_ids=list(range(B)),
                               trace=_trace)
    kernel.last_results = res
    out = np.stack([np.asarray(res.results[i]["out"]) for i in range(B)])
    return out.reshape(B, H, W, C).astype(np.float32)
